# revision 8
# baseline (speedup 1.0000x reference)
"""Trainium2 Bass kernel for nn_Attention_73770358276185 — v4.

Per-batch computation (B=8, one batch per NeuronCore, data-parallel):
    f = gelu(BN(Wf @ q + bf))            [64, 4096]
    g = gelu(BN(Wg @ k + bg))            [64, 4096]
    h = gelu(BN(Wh @ k + bh))            [256, 4096]
    s[i,j] = sum_l g[l,i] f[l,j]         [4096, 4096]
    beta = softmax_j(s)
    o[i,c] = sum_j beta[i,j] h[c,j]
    out = gamma * o.T + q

v4 structure:
  - all weights/constants packed into ONE DMA; k/q stream in pairwise
    512-col chunks so projections start ~immediately
  - projection chains emitted in data-arrival order (g, h-pairs, f per
    chunk); h processed in [128,512] pair tiles with a single strided-out
    gelu per pair
  - mm1 contracts K=64 directly (no zero-padding); softmax unshifted
    (exp stays in fp32 range); ones-column of h_aug gives the row sum
  - epilogue: bf16 osc, DMA-xbar transposes (no PE transposes), ib-major
    last group so o_aug PSUM slots free incrementally
"""
import sys

for _p in ("/opt/trn_rl_repo", "/root/.axon_site/_ro/trn_rl_repo"):
    if _p not in sys.path:
        sys.path.insert(0, _p)

import numpy as np

import concourse.bacc as bacc
import concourse.tile as tile
import concourse.mybir as mybir
from concourse.bass_utils import run_bass_kernel_spmd

P = 128
B = 8
N = 4096          # sequence positions
C1 = 256          # dim1 (q channels / h channels)
C2 = 128          # dim2 (k channels)
L = 64            # layer = dim1 // 4 (f/g channels)
EPS = 1e-5

NJB = N // P      # 32 j-blocks
NIC = 8           # i chunks
IC = N // NIC     # 512 columns per i chunk
JG = 2            # j-blocks per exp group
NGRP = NJB // JG  # 16 groups
NKC = 8           # k/q load + projection chunks
KC = N // NKC     # 512 cols per chunk
HST = 258         # h_aug row stride (256 ch + ones col + pad; even for f32r)

# packed-constant column offsets: wg|wh first (needed by the earliest
# projections, shipped in DMA A), then wf0|wf1|dht2|ident|gm|df|dg (DMA B)
OWG, OWH = 0, L
OWF0 = OWH + C1
OWF1 = OWF0 + L
ODH = OWF1 + L
OID = ODH + 2 * C1
OGM = OID + P
ODF, ODG = OGM + 1, OGM + 2
WPK = OGM + 4  # 1092 cols, padded even
WSPLIT = OWF0  # DMA A covers [0, WSPLIT), DMA B the rest

F32 = mybir.dt.float32
F32R = mybir.dt.float32r
BF16 = mybir.dt.bfloat16
AF = mybir.ActivationFunctionType
MUL = mybir.AluOpType.mult

_BUILT = None  # (nc) cache — the program is input-value independent


def _round_tf32(x):
    """Round fp32 to float32r (drop 12 mantissa bits, round-to-nearest)."""
    v = np.ascontiguousarray(x, dtype=np.float32).view(np.uint32).astype(np.uint64)
    half = np.uint64(0x7FF)
    lsb = (v >> np.uint64(12)) & np.uint64(1)
    v = (v + half + lsb) & np.uint64(0xFFFFF000)
    return v.astype(np.uint32).view(np.float32)


def _build(repeat=1, loads_in_loop=False):
    nc = bacc.Bacc("TRN2", target_bir_lowering=False, debug=False)

    q2r = nc.dram_tensor("q2r", [C1, N], F32R, kind="ExternalInput")  # tf32 q
    k2r = nc.dram_tensor("k2r", [C2, N], F32R, kind="ExternalInput")
    wpack = nc.dram_tensor("wpack", [P, WPK], F32R, kind="ExternalInput")
    o_out = nc.dram_tensor("o_out", [C1, N], F32, kind="ExternalOutput")

    with tile.TileContext(nc) as tc:
        with (
            tc.tile_pool(name="const", bufs=1) as cp,
            tc.tile_pool(name="acc", bufs=4, space="PSUM") as accp,
            tc.tile_pool(name="sT", bufs=2, space="PSUM") as sTp,
            tc.tile_pool(name="ex", bufs=4) as exp_,
            tc.tile_pool(name="osc", bufs=4) as oscp,
            tc.tile_pool(name="ott", bufs=4) as ottp,
            tc.tile_pool(name="rin", bufs=4) as rinp,
            tc.tile_pool(name="outst", bufs=4) as outp,
        ):
            if not loads_in_loop:
                env0 = _emit_loads(nc, tc, locals())
            else:
                env0 = None

            import contextlib
            loop_cm = tc.For_i(0, repeat, 1) if repeat > 1 else contextlib.nullcontext()
            with loop_cm:
                _env = dict(locals())
                if loads_in_loop:
                    env0 = _emit_loads(nc, tc, _env)
                _env.update(env0)
                _emit_body(nc, tc, _env)

    nc.finalize()
    return nc


def _emit_loads(nc, tc, env):
    """One packed constant DMA, then pairwise-interleaved k/q chunk loads."""
    cp = env["cp"]
    k2r = env["k2r"]; q2r = env["q2r"]; wpack = env["wpack"]

    wp = cp.tile([P, WPK], F32R, tag="wp")
    nc.sync.dma_start(wp[:], wpack[:, :])

    k_sb = cp.tile([C2, N], F32R, tag="k")
    q_sb = [cp.tile([P, N], F32R, tag=f"q{cb}", name=f"q{cb}") for cb in range(2)]
    for c in range(NKC):
        sl = slice(c * KC, (c + 1) * KC)
        nc.sync.dma_start(k_sb[:, sl], k2r[:, sl])
        for cb in range(2):
            nc.sync.dma_start(q_sb[cb][:, sl], q2r[cb * P:(cb + 1) * P, sl])

    return dict(
        k_sb=k_sb, q_sb=q_sb,
        wf=[wp[:, OWF0:OWF0 + L], wp[:, OWF1:OWF1 + L]],
        wg=wp[0:C2, OWG:OWG + L],
        wh=wp[0:C2, OWH:OWH + C1],
        dht2=wp[:, ODH:ODH + 2 * C1].bitcast(F32),
        idt=wp[:, OID:OID + P],
        gm=wp[:, OGM:OGM + 1].bitcast(F32),
        dft=wp[0:L, ODF:ODF + 1].bitcast(F32),
        dgt=wp[0:L, ODG:ODG + 1].bitcast(F32),
    )


def _emit_body(nc, tc, env):
    accp = env["accp"]; sTp = env["sTp"]; exp_ = env["exp_"]
    oscp = env["oscp"]; ottp = env["ottp"]; rinp = env["rinp"]
    outp = env["outp"]; cp = env["cp"]
    k_sb = env["k_sb"]; q_sb = env["q_sb"]
    wf = env["wf"]; wg = env["wg"]; wh = env["wh"]
    dft = env["dft"]; dgt = env["dgt"]; dht2 = env["dht2"]; gm = env["gm"]
    idt = env["idt"]; o_out = env["o_out"]

    f_sb = cp.tile([L, N], F32R, tag="f")
    g_sb = cp.tile([L, N], F32R, tag="g")
    h_aug = cp.tile([P, NJB * HST + C1], F32R, tag="h")

    # ones columns of h_aug (col 256 = softmax-denominator dot; col 257 pad)
    ones_cols = (h_aug[:, 0:NJB * HST]
                 .rearrange("p (j c) -> p j c", j=NJB)[:, :, C1:HST]
                 .bitcast(F32))
    nc.vector.memset(ones_cols, 1.0)

    def emit_h_quad(hq):
        """Project h for j-blocks 4hq..4hq+3 in one [128,1024] PSUM tile
        (borrows the sT pool's 2-bank slots, idle during projections)."""
        jb0 = 4 * hq
        ps = sTp.tile([P, 4 * C1], F32, tag="sT", name="hps")
        for t in range(4):
            nc.tensor.matmul(ps[:, t * C1:(t + 1) * C1],
                             k_sb[:, (jb0 + t) * P:(jb0 + t + 1) * P], wh,
                             start=True, stop=True)
        for t in range(2):
            nc.vector.tensor_add(ps[:, 2 * t * C1:2 * (t + 1) * C1],
                                 ps[:, 2 * t * C1:2 * (t + 1) * C1], dht2)
        out4 = (h_aug[:, jb0 * HST:(jb0 + 4) * HST]
                .rearrange("p (t c) -> p t c", t=4)[:, :, 0:C1])
        nc.scalar.activation(out4, ps[:].rearrange("p (t c) -> p t c", t=4),
                             AF.Gelu)

    def emit_g(c):
        ps = accp.tile([L, KC], F32, tag="acc", name="gps")
        nc.tensor.matmul(ps[:], wg, k_sb[:, c * KC:(c + 1) * KC],
                         start=True, stop=True)
        nc.scalar.activation(g_sb[:, c * KC:(c + 1) * KC], ps[:],
                             AF.Gelu, bias=dgt)

    def emit_f(c):
        ps = accp.tile([L, KC], F32, tag="acc", name="fps")
        nc.tensor.matmul(ps[:], wf[0], q_sb[0][:, c * KC:(c + 1) * KC],
                         start=True, stop=False)
        nc.tensor.matmul(ps[:], wf[1], q_sb[1][:, c * KC:(c + 1) * KC],
                         start=False, stop=True)
        nc.scalar.activation(f_sb[:, c * KC:(c + 1) * KC], ps[:],
                             AF.Gelu, bias=dft)

    # ---- projections in data-arrival order -------------------------------
    for c in range(NKC):
        emit_g(c)
        emit_h_quad(c)
        emit_f(c)

    # ---- attention main loop (software-pipelined emission) ---------------
    o_augs = {}

    def emit_mm1(ic, grp):
        sT = sTp.tile([P, JG * IC], F32, tag="sT", name="sT")
        for t in range(JG):
            jb = grp * JG + t
            nc.tensor.matmul(sT[:, t * IC:(t + 1) * IC],
                             f_sb[:, jb * P:(jb + 1) * P],
                             g_sb[:, ic * IC:(ic + 1) * IC],
                             start=True, stop=True)
        ex = exp_.tile([P, JG * IC], F32R, tag="ex", name="ex")
        nc.scalar.activation(ex[:], sT[:], AF.Exp)
        return ex

    def emit_mm2(ic, grp, ex):
        if grp == 0:
            o_augs[ic] = [
                accp.tile([P, HST], F32, tag="acc", name=f"oaug{ib}")
                for ib in range(4)]
        o_aug = o_augs[ic]
        for t in range(JG):
            jb = grp * JG + t
            for ib in range(4):
                nc.tensor.matmul(
                    o_aug[ib][:],
                    ex[:, t * IC + ib * P:t * IC + (ib + 1) * P],
                    h_aug[:, jb * HST:(jb + 1) * HST],
                    start=(grp == 0 and t == 0),
                    stop=False)

    def emit_mm2_last(ic, ex):
        """Last group of an i-chunk: ib-major order with the per-ib epilogue
        fused in, so each o_aug PSUM slot frees as soon as it's consumed."""
        grp = NGRP - 1
        o_aug = o_augs.pop(ic)
        ost = [outp.tile([P, IC], F32, tag=f"ost{cb}", name=f"ost{cb}")
               for cb in range(2)]
        for ib in range(4):
            for t in range(JG):
                jb = grp * JG + t
                nc.tensor.matmul(
                    o_aug[ib][:],
                    ex[:, t * IC + ib * P:t * IC + (ib + 1) * P],
                    h_aug[:, jb * HST:(jb + 1) * HST],
                    start=False, stop=(t == JG - 1))
            rv = rinp.tile([P, 1], F32, tag="rin", name="rv")
            nc.vector.reciprocal(rv[:], o_aug[ib][:, C1:C1 + 1])
            osc = oscp.tile([P, C1], F32R, tag="osc", name="osc")
            nc.vector.tensor_scalar(osc[:], o_aug[ib][:, 0:C1], rv[:],
                                    gm, op0=MUL, op1=MUL)
            for cb in range(2):
                # transpose back into the dying o_aug bank (no extra PSUM)
                oT = o_aug[ib][:, cb * P:(cb + 1) * P]
                nc.tensor.transpose(oT.bitcast(F32R),
                                    osc[:, cb * P:(cb + 1) * P], idt)
                nc.vector.tensor_add(
                    ost[cb][:, ib * P:(ib + 1) * P], oT,
                    q_sb[cb][:, ic * IC + ib * P:ic * IC + (ib + 1) * P]
                    .bitcast(F32))
        for cb in range(2):
            nc.sync.dma_start(
                o_out[cb * P:(cb + 1) * P, ic * IC:(ic + 1) * IC],
                ost[cb][:])

    groups = [(ic, grp) for ic in range(NIC) for grp in range(NGRP)]
    pending = None  # (ic, grp, ex) whose mm2 is not yet emitted
    for (ic, grp) in groups:
        ex = emit_mm1(ic, grp)
        if pending is not None:
            pic, pgrp, pex = pending
            if pgrp == NGRP - 1:
                emit_mm2_last(pic, pex)
            else:
                emit_mm2(pic, pgrp, pex)
        pending = (ic, grp, ex)
    pic, pgrp, pex = pending
    emit_mm2_last(pic, pex)


def _preprocess(inputs):
    """Fold conv bias + BN into effective weights/biases, per-core input maps."""
    f32 = np.float32
    q = np.ascontiguousarray(inputs["q"], dtype=f32)[..., 0]   # [B, 256, N]
    k = np.ascontiguousarray(inputs["k"], dtype=f32)[..., 0]   # [B, 128, N]

    def fold(W, b, scale, bias, mean, var):
        inv = (np.asarray(scale, f32) /
               np.sqrt(np.asarray(var, f32) + f32(EPS))).astype(f32)
        W_eff = (inv[:, None] * np.asarray(W, f32)).astype(f32)
        delta = ((np.asarray(b, f32) - np.asarray(mean, f32)) * inv
                 + np.asarray(bias, f32)).astype(f32)
        return W_eff, delta

    Wf_e, d_f = fold(inputs["Wf"], inputs["bf"], inputs["fs"], inputs["fb"],
                     inputs["fm"], inputs["fv"])
    Wg_e, d_g = fold(inputs["Wg"], inputs["bg"], inputs["gs"], inputs["gb"],
                     inputs["gm"], inputs["gv"])
    Wh_e, d_h = fold(inputs["Wh"], inputs["bh"], inputs["hs"], inputs["hb"],
                     inputs["hm"], inputs["hv"])

    gamma = f32(np.asarray(inputs["gamma"], f32).reshape(-1)[0])
    wpack = np.zeros((P, WPK), f32)
    wpack[:, OWF0:OWF0 + L] = _round_tf32(Wf_e.T[0:P])
    wpack[:, OWF1:OWF1 + L] = _round_tf32(Wf_e.T[P:C1])
    wpack[0:C2, OWG:OWG + L] = _round_tf32(Wg_e.T)
    wpack[0:C2, OWH:OWH + C1] = _round_tf32(Wh_e.T)
    wpack[:, ODH:ODH + 2 * C1] = np.broadcast_to(
        np.tile(d_h, 2), (P, 2 * C1))
    wpack[:, OID:OID + P] = np.eye(P, dtype=f32)
    wpack[:, OGM] = gamma
    wpack[0:L, ODF] = d_f
    wpack[0:L, ODG] = d_g

    in_maps = []
    for b_ in range(B):
        m = {"wpack": wpack}
        m["q2r"] = _round_tf32(q[b_])
        m["k2r"] = _round_tf32(k[b_])
        in_maps.append(m)
    return in_maps


def _get_nc():
    global _BUILT
    if _BUILT is None:
        _BUILT = _build()
    return _BUILT


def kernel(**inputs):
    nc = _get_nc()
    in_maps = _preprocess(inputs)
    res = run_bass_kernel_spmd(nc, in_maps, core_ids=list(range(B)))
    out = np.stack([res.results[i]["o_out"] for i in range(B)])
    return out[..., None].astype(np.float32)


# revision 9
# speedup vs baseline: 1.0017x; 1.0017x over previous
"""Trainium2 Bass kernel for nn_Attention_73770358276185 — v4.

Per-batch computation (B=8, one batch per NeuronCore, data-parallel):
    f = gelu(BN(Wf @ q + bf))            [64, 4096]
    g = gelu(BN(Wg @ k + bg))            [64, 4096]
    h = gelu(BN(Wh @ k + bh))            [256, 4096]
    s[i,j] = sum_l g[l,i] f[l,j]         [4096, 4096]
    beta = softmax_j(s)
    o[i,c] = sum_j beta[i,j] h[c,j]
    out = gamma * o.T + q

v4 structure:
  - all weights/constants packed into ONE DMA; k/q stream in pairwise
    512-col chunks so projections start ~immediately
  - projection chains emitted in data-arrival order (g, h-pairs, f per
    chunk); h processed in [128,512] pair tiles with a single strided-out
    gelu per pair
  - mm1 contracts K=64 directly (no zero-padding); softmax unshifted
    (exp stays in fp32 range); ones-column of h_aug gives the row sum
  - epilogue: bf16 osc, DMA-xbar transposes (no PE transposes), ib-major
    last group so o_aug PSUM slots free incrementally
"""
import sys

for _p in ("/opt/trn_rl_repo", "/root/.axon_site/_ro/trn_rl_repo"):
    if _p not in sys.path:
        sys.path.insert(0, _p)

import numpy as np

import concourse.bacc as bacc
import concourse.tile as tile
import concourse.mybir as mybir
from concourse.bass_utils import run_bass_kernel_spmd

P = 128
B = 8
N = 4096          # sequence positions
C1 = 256          # dim1 (q channels / h channels)
C2 = 128          # dim2 (k channels)
L = 64            # layer = dim1 // 4 (f/g channels)
EPS = 1e-5

NJB = N // P      # 32 j-blocks
NIC = 8           # i chunks
IC = N // NIC     # 512 columns per i chunk
JG = 2            # j-blocks per exp group
NGRP = NJB // JG  # 16 groups
NKC = 8           # k/q load + projection chunks
KC = N // NKC     # 512 cols per chunk
HST = 258         # h_aug row stride (256 ch + ones col + pad; even for f32r)

# packed-constant column offsets: wg|wh first (needed by the earliest
# projections, shipped in DMA A), then wf0|wf1|dht2|ident|gm|df|dg (DMA B)
OWG, OWH = 0, L
OWF0 = OWH + C1
OWF1 = OWF0 + L
ODH = OWF1 + L
OID = ODH + 2 * C1
OGM = OID + P
ODF, ODG = OGM + 1, OGM + 2
WPK = OGM + 4  # 1092 cols, padded even
WSPLIT = OWF0  # DMA A covers [0, WSPLIT), DMA B the rest

F32 = mybir.dt.float32
F32R = mybir.dt.float32r
BF16 = mybir.dt.bfloat16
AF = mybir.ActivationFunctionType
MUL = mybir.AluOpType.mult

_BUILT = None  # (nc) cache — the program is input-value independent


def _round_tf32(x):
    """Round fp32 to float32r (drop 12 mantissa bits, round-to-nearest)."""
    v = np.ascontiguousarray(x, dtype=np.float32).view(np.uint32).astype(np.uint64)
    half = np.uint64(0x7FF)
    lsb = (v >> np.uint64(12)) & np.uint64(1)
    v = (v + half + lsb) & np.uint64(0xFFFFF000)
    return v.astype(np.uint32).view(np.float32)


def _build(repeat=1, loads_in_loop=False):
    nc = bacc.Bacc("TRN2", target_bir_lowering=False, debug=False)

    q2r = nc.dram_tensor("q2r", [C1, N], F32R, kind="ExternalInput")  # tf32 q
    k2r = nc.dram_tensor("k2r", [C2, N], F32R, kind="ExternalInput")
    wpack = nc.dram_tensor("wpack", [P, WPK], F32R, kind="ExternalInput")
    o_out = nc.dram_tensor("o_out", [C1, N], F32, kind="ExternalOutput")

    with tile.TileContext(nc) as tc:
        with (
            tc.tile_pool(name="const", bufs=1) as cp,
            tc.tile_pool(name="acc", bufs=4, space="PSUM") as accp,
            tc.tile_pool(name="sT", bufs=2, space="PSUM") as sTp,
            tc.tile_pool(name="ex", bufs=4) as exp_,
            tc.tile_pool(name="osc", bufs=2) as oscp,
            tc.tile_pool(name="ott", bufs=4) as ottp,
            tc.tile_pool(name="rin", bufs=4) as rinp,
            tc.tile_pool(name="outst", bufs=4) as outp,
        ):
            if not loads_in_loop:
                env0 = _emit_loads(nc, tc, locals())
            else:
                env0 = None

            import contextlib
            loop_cm = tc.For_i(0, repeat, 1) if repeat > 1 else contextlib.nullcontext()
            with loop_cm:
                _env = dict(locals())
                if loads_in_loop:
                    env0 = _emit_loads(nc, tc, _env)
                _env.update(env0)
                _emit_body(nc, tc, _env)

    nc.finalize()
    return nc


def _emit_loads(nc, tc, env):
    """One packed constant DMA, then pairwise-interleaved k/q chunk loads."""
    cp = env["cp"]
    k2r = env["k2r"]; q2r = env["q2r"]; wpack = env["wpack"]

    wp = cp.tile([P, WPK], F32R, tag="wp")
    nc.sync.dma_start(wp[:], wpack[:, :])

    k_sb = cp.tile([C2, N], F32R, tag="k")
    q_sb = [cp.tile([P, N], F32R, tag=f"q{cb}", name=f"q{cb}") for cb in range(2)]
    for c in range(NKC):
        sl = slice(c * KC, (c + 1) * KC)
        nc.sync.dma_start(k_sb[:, sl], k2r[:, sl])
        for cb in range(2):
            nc.sync.dma_start(q_sb[cb][:, sl], q2r[cb * P:(cb + 1) * P, sl])

    return dict(
        k_sb=k_sb, q_sb=q_sb,
        wf=[wp[:, OWF0:OWF0 + L], wp[:, OWF1:OWF1 + L]],
        wg=wp[0:C2, OWG:OWG + L],
        wh=wp[0:C2, OWH:OWH + C1],
        dht2=wp[:, ODH:ODH + 2 * C1].bitcast(F32),
        idt=wp[:, OID:OID + P],
        gm=wp[:, OGM:OGM + 1].bitcast(F32),
        dft=wp[0:L, ODF:ODF + 1].bitcast(F32),
        dgt=wp[0:L, ODG:ODG + 1].bitcast(F32),
    )


def _emit_body(nc, tc, env):
    accp = env["accp"]; sTp = env["sTp"]; exp_ = env["exp_"]
    oscp = env["oscp"]; ottp = env["ottp"]; rinp = env["rinp"]
    outp = env["outp"]; cp = env["cp"]
    k_sb = env["k_sb"]; q_sb = env["q_sb"]
    wf = env["wf"]; wg = env["wg"]; wh = env["wh"]
    dft = env["dft"]; dgt = env["dgt"]; dht2 = env["dht2"]; gm = env["gm"]
    idt = env["idt"]; o_out = env["o_out"]

    f_sb = cp.tile([L, N], F32R, tag="f")
    g_sb = cp.tile([L, N], F32R, tag="g")
    h_aug = cp.tile([P, NJB * HST + C1], F32R, tag="h")

    # ones columns of h_aug (col 256 = softmax-denominator dot; col 257 pad)
    ones_cols = (h_aug[:, 0:NJB * HST]
                 .rearrange("p (j c) -> p j c", j=NJB)[:, :, C1:HST]
                 .bitcast(F32))
    nc.vector.memset(ones_cols, 1.0)

    def emit_h_quad(hq):
        """Project h for j-blocks 4hq..4hq+3 in one [128,1024] PSUM tile
        (borrows the sT pool's 2-bank slots, idle during projections)."""
        jb0 = 4 * hq
        ps = sTp.tile([P, 4 * C1], F32, tag="sT", name="hps")
        for t in range(4):
            nc.tensor.matmul(ps[:, t * C1:(t + 1) * C1],
                             k_sb[:, (jb0 + t) * P:(jb0 + t + 1) * P], wh,
                             start=True, stop=True)
        for t in range(2):
            nc.vector.tensor_add(ps[:, 2 * t * C1:2 * (t + 1) * C1],
                                 ps[:, 2 * t * C1:2 * (t + 1) * C1], dht2)
        out4 = (h_aug[:, jb0 * HST:(jb0 + 4) * HST]
                .rearrange("p (t c) -> p t c", t=4)[:, :, 0:C1])
        nc.scalar.activation(out4, ps[:].rearrange("p (t c) -> p t c", t=4),
                             AF.Gelu)

    def emit_g(c):
        ps = accp.tile([L, KC], F32, tag="acc", name="gps")
        nc.tensor.matmul(ps[:], wg, k_sb[:, c * KC:(c + 1) * KC],
                         start=True, stop=True)
        nc.scalar.activation(g_sb[:, c * KC:(c + 1) * KC], ps[:],
                             AF.Gelu, bias=dgt)

    def emit_f(c):
        ps = accp.tile([L, KC], F32, tag="acc", name="fps")
        nc.tensor.matmul(ps[:], wf[0], q_sb[0][:, c * KC:(c + 1) * KC],
                         start=True, stop=False)
        nc.tensor.matmul(ps[:], wf[1], q_sb[1][:, c * KC:(c + 1) * KC],
                         start=False, stop=True)
        nc.scalar.activation(f_sb[:, c * KC:(c + 1) * KC], ps[:],
                             AF.Gelu, bias=dft)

    # ---- projections in data-arrival order -------------------------------
    for c in range(NKC):
        emit_g(c)
        emit_h_quad(c)
        emit_f(c)

    # ---- attention main loop (software-pipelined emission) ---------------
    o_augs = {}

    def emit_mm1(ic, grp):
        sT = sTp.tile([P, JG * IC], F32, tag="sT", name="sT")
        for t in range(JG):
            jb = grp * JG + t
            nc.tensor.matmul(sT[:, t * IC:(t + 1) * IC],
                             f_sb[:, jb * P:(jb + 1) * P],
                             g_sb[:, ic * IC:(ic + 1) * IC],
                             start=True, stop=True)
        ex = exp_.tile([P, JG * IC], F32R, tag="ex", name="ex")
        nc.scalar.activation(ex[:], sT[:], AF.Exp)
        return ex

    def emit_mm2(ic, grp, ex):
        if grp == 0:
            o_augs[ic] = [
                accp.tile([P, HST], F32, tag="acc", name=f"oaug{ib}")
                for ib in range(4)]
        o_aug = o_augs[ic]
        for t in range(JG):
            jb = grp * JG + t
            for ib in range(4):
                nc.tensor.matmul(
                    o_aug[ib][:],
                    ex[:, t * IC + ib * P:t * IC + (ib + 1) * P],
                    h_aug[:, jb * HST:(jb + 1) * HST],
                    start=(grp == 0 and t == 0),
                    stop=False)

    def emit_mm2_last(ic, ex):
        """Last group of an i-chunk: ib-major order with the per-ib epilogue
        fused in, so each o_aug PSUM slot frees as soon as it's consumed."""
        grp = NGRP - 1
        o_aug = o_augs.pop(ic)
        ost = [outp.tile([P, IC], F32, tag=f"ost{cb}", name=f"ost{cb}")
               for cb in range(2)]
        for ib in range(4):
            for t in range(JG):
                jb = grp * JG + t
                nc.tensor.matmul(
                    o_aug[ib][:],
                    ex[:, t * IC + ib * P:t * IC + (ib + 1) * P],
                    h_aug[:, jb * HST:(jb + 1) * HST],
                    start=False, stop=(t == JG - 1))
            rv = rinp.tile([P, 1], F32, tag="rin", name="rv")
            nc.vector.reciprocal(rv[:], o_aug[ib][:, C1:C1 + 1])
            osc = oscp.tile([P, C1], F32R, tag="osc", name="osc")
            nc.vector.tensor_scalar(osc[:], o_aug[ib][:, 0:C1], rv[:],
                                    gm, op0=MUL, op1=MUL)
            for cb in range(2):
                # transpose back into the dying o_aug bank (no extra PSUM)
                oT = o_aug[ib][:, cb * P:(cb + 1) * P]
                nc.tensor.transpose(oT.bitcast(F32R),
                                    osc[:, cb * P:(cb + 1) * P], idt)
                nc.vector.tensor_add(
                    ost[cb][:, ib * P:(ib + 1) * P], oT,
                    q_sb[cb][:, ic * IC + ib * P:ic * IC + (ib + 1) * P]
                    .bitcast(F32))
        for cb in range(2):
            nc.sync.dma_start(
                o_out[cb * P:(cb + 1) * P, ic * IC:(ic + 1) * IC],
                ost[cb][:])

    groups = [(ic, grp) for ic in range(NIC) for grp in range(NGRP)]
    pending = None  # (ic, grp, ex) whose mm2 is not yet emitted
    for (ic, grp) in groups:
        ex = emit_mm1(ic, grp)
        if pending is not None:
            pic, pgrp, pex = pending
            if pgrp == NGRP - 1:
                emit_mm2_last(pic, pex)
            else:
                emit_mm2(pic, pgrp, pex)
        pending = (ic, grp, ex)
    pic, pgrp, pex = pending
    emit_mm2_last(pic, pex)


def _preprocess(inputs):
    """Fold conv bias + BN into effective weights/biases, per-core input maps."""
    f32 = np.float32
    q = np.ascontiguousarray(inputs["q"], dtype=f32)[..., 0]   # [B, 256, N]
    k = np.ascontiguousarray(inputs["k"], dtype=f32)[..., 0]   # [B, 128, N]

    def fold(W, b, scale, bias, mean, var):
        inv = (np.asarray(scale, f32) /
               np.sqrt(np.asarray(var, f32) + f32(EPS))).astype(f32)
        W_eff = (inv[:, None] * np.asarray(W, f32)).astype(f32)
        delta = ((np.asarray(b, f32) - np.asarray(mean, f32)) * inv
                 + np.asarray(bias, f32)).astype(f32)
        return W_eff, delta

    Wf_e, d_f = fold(inputs["Wf"], inputs["bf"], inputs["fs"], inputs["fb"],
                     inputs["fm"], inputs["fv"])
    Wg_e, d_g = fold(inputs["Wg"], inputs["bg"], inputs["gs"], inputs["gb"],
                     inputs["gm"], inputs["gv"])
    Wh_e, d_h = fold(inputs["Wh"], inputs["bh"], inputs["hs"], inputs["hb"],
                     inputs["hm"], inputs["hv"])

    gamma = f32(np.asarray(inputs["gamma"], f32).reshape(-1)[0])
    wpack = np.zeros((P, WPK), f32)
    wpack[:, OWF0:OWF0 + L] = _round_tf32(Wf_e.T[0:P])
    wpack[:, OWF1:OWF1 + L] = _round_tf32(Wf_e.T[P:C1])
    wpack[0:C2, OWG:OWG + L] = _round_tf32(Wg_e.T)
    wpack[0:C2, OWH:OWH + C1] = _round_tf32(Wh_e.T)
    wpack[:, ODH:ODH + 2 * C1] = np.broadcast_to(
        np.tile(d_h, 2), (P, 2 * C1))
    wpack[:, OID:OID + P] = np.eye(P, dtype=f32)
    wpack[:, OGM] = gamma
    wpack[0:L, ODF] = d_f
    wpack[0:L, ODG] = d_g

    in_maps = []
    for b_ in range(B):
        m = {"wpack": wpack}
        m["q2r"] = _round_tf32(q[b_])
        m["k2r"] = _round_tf32(k[b_])
        in_maps.append(m)
    return in_maps


def _get_nc():
    global _BUILT
    if _BUILT is None:
        _BUILT = _build()
    return _BUILT


def kernel(**inputs):
    nc = _get_nc()
    in_maps = _preprocess(inputs)
    res = run_bass_kernel_spmd(nc, in_maps, core_ids=list(range(B)))
    out = np.stack([res.results[i]["o_out"] for i in range(B)])
    return out[..., None].astype(np.float32)


# revision 11
# speedup vs baseline: 1.0068x; 1.0051x over previous
"""Trainium2 Bass kernel for nn_Attention_73770358276185 — v4.

Per-batch computation (B=8, one batch per NeuronCore, data-parallel):
    f = gelu(BN(Wf @ q + bf))            [64, 4096]
    g = gelu(BN(Wg @ k + bg))            [64, 4096]
    h = gelu(BN(Wh @ k + bh))            [256, 4096]
    s[i,j] = sum_l g[l,i] f[l,j]         [4096, 4096]
    beta = softmax_j(s)
    o[i,c] = sum_j beta[i,j] h[c,j]
    out = gamma * o.T + q

v4 structure:
  - all weights/constants packed into ONE DMA; k/q stream in pairwise
    512-col chunks so projections start ~immediately
  - projection chains emitted in data-arrival order (g, h-pairs, f per
    chunk); h processed in [128,512] pair tiles with a single strided-out
    gelu per pair
  - mm1 contracts K=64 directly (no zero-padding); softmax unshifted
    (exp stays in fp32 range); ones-column of h_aug gives the row sum
  - epilogue: bf16 osc, DMA-xbar transposes (no PE transposes), ib-major
    last group so o_aug PSUM slots free incrementally
"""
import sys

for _p in ("/opt/trn_rl_repo", "/root/.axon_site/_ro/trn_rl_repo"):
    if _p not in sys.path:
        sys.path.insert(0, _p)

import numpy as np

import concourse.bacc as bacc
import concourse.tile as tile
import concourse.mybir as mybir
from concourse.bass_utils import run_bass_kernel_spmd

P = 128
B = 8
N = 4096          # sequence positions
C1 = 256          # dim1 (q channels / h channels)
C2 = 128          # dim2 (k channels)
L = 64            # layer = dim1 // 4 (f/g channels)
EPS = 1e-5

NJB = N // P      # 32 j-blocks
NIC = 8           # i chunks
IC = N // NIC     # 512 columns per i chunk
JG = 2            # j-blocks per exp group
NGRP = NJB // JG  # 16 groups
NKC = 8           # k/q load + projection chunks
KC = N // NKC     # 512 cols per chunk
HST = 258         # h_aug row stride (256 ch + ones col + pad; fp32r matmuls
                  # require an even moving width — s3d3_mm_fp32r_restrictions)

# packed-constant column offsets: wg|wh first (needed by the earliest
# projections, shipped in DMA A), then wf0|wf1|dht2|ident|gm|df|dg (DMA B)
OWG, OWH = 0, L
OWF0 = OWH + C1
OWF1 = OWF0 + L
ODH = OWF1 + L
OID = ODH + 2 * C1   # bf16 identity packed as 64 f32 columns
OGM = OID + P // 2
ODF, ODG = OGM + 1, OGM + 2
WPK = OGM + 4  # 1028 cols, padded even
WSPLIT = OWF0  # DMA A covers [0, WSPLIT), DMA B the rest

F32 = mybir.dt.float32
F32R = mybir.dt.float32r
BF16 = mybir.dt.bfloat16
AF = mybir.ActivationFunctionType
MUL = mybir.AluOpType.mult

_BUILT = None  # (nc) cache — the program is input-value independent


def _round_tf32(x):
    """Round fp32 to float32r (drop 12 mantissa bits, round-to-nearest)."""
    v = np.ascontiguousarray(x, dtype=np.float32).view(np.uint32).astype(np.uint64)
    half = np.uint64(0x7FF)
    lsb = (v >> np.uint64(12)) & np.uint64(1)
    v = (v + half + lsb) & np.uint64(0xFFFFF000)
    return v.astype(np.uint32).view(np.float32)


def _build(repeat=1, loads_in_loop=False):
    nc = bacc.Bacc("TRN2", target_bir_lowering=False, debug=False)

    q2r = nc.dram_tensor("q2r", [C1, N], F32R, kind="ExternalInput")  # tf32 q
    k2r = nc.dram_tensor("k2r", [C2, N], F32R, kind="ExternalInput")
    wpack = nc.dram_tensor("wpack", [P, WPK], F32R, kind="ExternalInput")
    o_out = nc.dram_tensor("o_out", [C1, N], F32, kind="ExternalOutput")

    with tile.TileContext(nc) as tc:
        with (
            tc.tile_pool(name="const", bufs=1) as cp,
            tc.tile_pool(name="acc", bufs=4, space="PSUM") as accp,
            tc.tile_pool(name="sT", bufs=2, space="PSUM") as sTp,
            tc.tile_pool(name="ex", bufs=4) as exp_,
            tc.tile_pool(name="osc", bufs=2) as oscp,
            tc.tile_pool(name="ott", bufs=4) as ottp,
            tc.tile_pool(name="rin", bufs=4) as rinp,
            tc.tile_pool(name="outst", bufs=4) as outp,
        ):
            if not loads_in_loop:
                env0 = _emit_loads(nc, tc, locals())
            else:
                env0 = None

            import contextlib
            loop_cm = tc.For_i(0, repeat, 1) if repeat > 1 else contextlib.nullcontext()
            with loop_cm:
                _env = dict(locals())
                if loads_in_loop:
                    env0 = _emit_loads(nc, tc, _env)
                _env.update(env0)
                _emit_body(nc, tc, _env)

    nc.finalize()
    return nc


def _emit_loads(nc, tc, env):
    """One packed constant DMA, then pairwise-interleaved k/q chunk loads."""
    cp = env["cp"]
    k2r = env["k2r"]; q2r = env["q2r"]; wpack = env["wpack"]

    wp = cp.tile([P, WPK], F32R, tag="wp")
    nc.sync.dma_start(wp[:], wpack[:, :])

    k_sb = cp.tile([C2, N], F32R, tag="k")
    q_sb = [cp.tile([P, N], F32R, tag=f"q{cb}", name=f"q{cb}") for cb in range(2)]
    for c in range(NKC):
        sl = slice(c * KC, (c + 1) * KC)
        nc.sync.dma_start(k_sb[:, sl], k2r[:, sl])
        for cb in range(2):
            nc.sync.dma_start(q_sb[cb][:, sl], q2r[cb * P:(cb + 1) * P, sl])

    return dict(
        k_sb=k_sb, q_sb=q_sb,
        wf=[wp[:, OWF0:OWF0 + L], wp[:, OWF1:OWF1 + L]],
        wg=wp[0:C2, OWG:OWG + L],
        wh=wp[0:C2, OWH:OWH + C1],
        dht2=wp[:, ODH:ODH + 2 * C1].bitcast(F32),
        idt=wp[:, OID:OID + P // 2].bitcast(BF16),
        gm=wp[:, OGM:OGM + 1].bitcast(F32),
        dft=wp[0:L, ODF:ODF + 1].bitcast(F32),
        dgt=wp[0:L, ODG:ODG + 1].bitcast(F32),
    )


def _emit_body(nc, tc, env):
    accp = env["accp"]; sTp = env["sTp"]; exp_ = env["exp_"]
    oscp = env["oscp"]; ottp = env["ottp"]; rinp = env["rinp"]
    outp = env["outp"]; cp = env["cp"]
    k_sb = env["k_sb"]; q_sb = env["q_sb"]
    wf = env["wf"]; wg = env["wg"]; wh = env["wh"]
    dft = env["dft"]; dgt = env["dgt"]; dht2 = env["dht2"]; gm = env["gm"]
    idt = env["idt"]; o_out = env["o_out"]

    f_sb = cp.tile([L, N], F32R, tag="f")
    g_sb = cp.tile([L, N], F32R, tag="g")
    h_aug = cp.tile([P, NJB * HST + C1], F32R, tag="h")

    # ones columns of h_aug (col 256 = softmax-denominator dot; col 257 pad)
    ones_cols = (h_aug[:, 0:NJB * HST]
                 .rearrange("p (j c) -> p j c", j=NJB)[:, :, C1:HST]
                 .bitcast(F32))
    nc.vector.memset(ones_cols, 1.0)

    def emit_h_quad(hq):
        """Project h for j-blocks 4hq..4hq+3 in one [128,1024] PSUM tile
        (borrows the sT pool's 2-bank slots, idle during projections)."""
        jb0 = 4 * hq
        ps = sTp.tile([P, 4 * C1], F32, tag="sT", name="hps")
        for t in range(4):
            nc.tensor.matmul(ps[:, t * C1:(t + 1) * C1],
                             k_sb[:, (jb0 + t) * P:(jb0 + t + 1) * P], wh,
                             start=True, stop=True)
        for t in range(2):
            nc.vector.tensor_add(ps[:, 2 * t * C1:2 * (t + 1) * C1],
                                 ps[:, 2 * t * C1:2 * (t + 1) * C1], dht2)
        out4 = (h_aug[:, jb0 * HST:(jb0 + 4) * HST]
                .rearrange("p (t c) -> p t c", t=4)[:, :, 0:C1])
        nc.scalar.activation(out4, ps[:].rearrange("p (t c) -> p t c", t=4),
                             AF.Gelu)

    def emit_g(c):
        ps = accp.tile([L, KC], F32, tag="acc", name="gps")
        nc.tensor.matmul(ps[:], wg, k_sb[:, c * KC:(c + 1) * KC],
                         start=True, stop=True)
        nc.scalar.activation(g_sb[:, c * KC:(c + 1) * KC], ps[:],
                             AF.Gelu, bias=dgt)

    def emit_f(c):
        ps = accp.tile([L, KC], F32, tag="acc", name="fps")
        nc.tensor.matmul(ps[:], wf[0], q_sb[0][:, c * KC:(c + 1) * KC],
                         start=True, stop=False)
        nc.tensor.matmul(ps[:], wf[1], q_sb[1][:, c * KC:(c + 1) * KC],
                         start=False, stop=True)
        nc.scalar.activation(f_sb[:, c * KC:(c + 1) * KC], ps[:],
                             AF.Gelu, bias=dft)

    # ---- projections in data-arrival order -------------------------------
    for c in range(NKC):
        emit_g(c)
        emit_h_quad(c)
        emit_f(c)

    # ---- attention main loop (software-pipelined emission) ---------------
    o_augs = {}

    def emit_mm1(ic, grp):
        sT = sTp.tile([P, JG * IC], F32, tag="sT", name="sT")
        for t in range(JG):
            jb = grp * JG + t
            nc.tensor.matmul(sT[:, t * IC:(t + 1) * IC],
                             f_sb[:, jb * P:(jb + 1) * P],
                             g_sb[:, ic * IC:(ic + 1) * IC],
                             start=True, stop=True)
        ex = exp_.tile([P, JG * IC], F32R, tag="ex", name="ex")
        nc.scalar.activation(ex[:], sT[:], AF.Exp)
        return ex

    def emit_mm2(ic, grp, ex):
        if grp == 0:
            o_augs[ic] = [
                accp.tile([P, HST], F32, tag="acc", name=f"oaug{ib}")
                for ib in range(4)]
        o_aug = o_augs[ic]
        for t in range(JG):
            jb = grp * JG + t
            for ib in range(4):
                nc.tensor.matmul(
                    o_aug[ib][:],
                    ex[:, t * IC + ib * P:t * IC + (ib + 1) * P],
                    h_aug[:, jb * HST:(jb + 1) * HST],
                    start=(grp == 0 and t == 0),
                    stop=False)

    def emit_mm2_last(ic, ex):
        """Last group of an i-chunk: ib-major order with the per-ib epilogue
        fused in, so each o_aug PSUM slot frees as soon as it's consumed."""
        grp = NGRP - 1
        o_aug = o_augs.pop(ic)
        ost = [outp.tile([P, IC], F32, tag=f"ost{cb}", name=f"ost{cb}")
               for cb in range(2)]
        for ib in range(4):
            for t in range(JG):
                jb = grp * JG + t
                nc.tensor.matmul(
                    o_aug[ib][:],
                    ex[:, t * IC + ib * P:t * IC + (ib + 1) * P],
                    h_aug[:, jb * HST:(jb + 1) * HST],
                    start=False, stop=(t == JG - 1))
            rv = rinp.tile([P, 1], F32, tag="rin", name="rv")
            nc.vector.reciprocal(rv[:], o_aug[ib][:, C1:C1 + 1])
            osc = oscp.tile([P, C1], BF16, tag="osc", name="osc")
            nc.vector.tensor_scalar(osc[:], o_aug[ib][:, 0:C1], rv[:],
                                    gm, op0=MUL, op1=MUL)
            for cb in range(2):
                # bf16 transpose (1 cyc/row) back into the dying o_aug bank
                oT = o_aug[ib][:, cb * P:cb * P + P // 2].bitcast(BF16)
                nc.tensor.transpose(oT, osc[:, cb * P:(cb + 1) * P], idt)
                nc.vector.tensor_add(
                    ost[cb][:, ib * P:(ib + 1) * P], oT,
                    q_sb[cb][:, ic * IC + ib * P:ic * IC + (ib + 1) * P]
                    .bitcast(F32))
        for cb in range(2):
            nc.sync.dma_start(
                o_out[cb * P:(cb + 1) * P, ic * IC:(ic + 1) * IC],
                ost[cb][:])

    groups = [(ic, grp) for ic in range(NIC) for grp in range(NGRP)]
    pending = None  # (ic, grp, ex) whose mm2 is not yet emitted
    for (ic, grp) in groups:
        ex = emit_mm1(ic, grp)
        if pending is not None:
            pic, pgrp, pex = pending
            if pgrp == NGRP - 1:
                emit_mm2_last(pic, pex)
            else:
                emit_mm2(pic, pgrp, pex)
        pending = (ic, grp, ex)
    pic, pgrp, pex = pending
    emit_mm2_last(pic, pex)


def _preprocess(inputs):
    """Fold conv bias + BN into effective weights/biases, per-core input maps."""
    f32 = np.float32
    q = np.ascontiguousarray(inputs["q"], dtype=f32)[..., 0]   # [B, 256, N]
    k = np.ascontiguousarray(inputs["k"], dtype=f32)[..., 0]   # [B, 128, N]

    def fold(W, b, scale, bias, mean, var):
        inv = (np.asarray(scale, f32) /
               np.sqrt(np.asarray(var, f32) + f32(EPS))).astype(f32)
        W_eff = (inv[:, None] * np.asarray(W, f32)).astype(f32)
        delta = ((np.asarray(b, f32) - np.asarray(mean, f32)) * inv
                 + np.asarray(bias, f32)).astype(f32)
        return W_eff, delta

    Wf_e, d_f = fold(inputs["Wf"], inputs["bf"], inputs["fs"], inputs["fb"],
                     inputs["fm"], inputs["fv"])
    Wg_e, d_g = fold(inputs["Wg"], inputs["bg"], inputs["gs"], inputs["gb"],
                     inputs["gm"], inputs["gv"])
    Wh_e, d_h = fold(inputs["Wh"], inputs["bh"], inputs["hs"], inputs["hb"],
                     inputs["hm"], inputs["hv"])

    gamma = f32(np.asarray(inputs["gamma"], f32).reshape(-1)[0])
    wpack = np.zeros((P, WPK), f32)
    wpack[:, OWF0:OWF0 + L] = _round_tf32(Wf_e.T[0:P])
    wpack[:, OWF1:OWF1 + L] = _round_tf32(Wf_e.T[P:C1])
    wpack[0:C2, OWG:OWG + L] = _round_tf32(Wg_e.T)
    wpack[0:C2, OWH:OWH + C1] = _round_tf32(Wh_e.T)
    wpack[:, ODH:ODH + 2 * C1] = np.broadcast_to(
        np.tile(d_h, 2), (P, 2 * C1))
    eye16 = np.zeros((P, P), np.uint16)
    eye16[np.arange(P), np.arange(P)] = 0x3F80  # bf16 1.0
    wpack[:, OID:OID + P // 2] = eye16.view(np.uint32).view(f32)
    wpack[:, OGM] = gamma
    wpack[0:L, ODF] = d_f
    wpack[0:L, ODG] = d_g

    in_maps = []
    for b_ in range(B):
        m = {"wpack": wpack}
        m["q2r"] = _round_tf32(q[b_])
        m["k2r"] = _round_tf32(k[b_])
        in_maps.append(m)
    return in_maps


def _get_nc():
    global _BUILT
    if _BUILT is None:
        _BUILT = _build()
    return _BUILT


def kernel(**inputs):
    nc = _get_nc()
    in_maps = _preprocess(inputs)
    res = run_bass_kernel_spmd(nc, in_maps, core_ids=list(range(B)))
    out = np.stack([res.results[i]["o_out"] for i in range(B)])
    return out[..., None].astype(np.float32)
